# revision 19
# baseline (speedup 1.0000x reference)
"""Trainium2 Bass kernel for GQA attention block (B=2, S=2048, H=2048,
16 q-heads / 4 kv-heads, head_dim=128, RoPE, causal) on 8 NeuronCores.

Sharding: core c -> batch b = c // 4, kv-group g = c % 4
  (q heads 4g..4g+3, kv head g).  Each core computes its batch's
  attention for its 4 query heads plus the partial output projection
  over its 512 hidden columns of w_o; host sums the 4 partials per batch.

Design (all matmuls bf16, PSUM accumulate f32):
  Phase 1  QKV projection in s-chunks of 512 (i-outer, kt-inner chains),
           RoPE via PE permutation matmul for rotate-half (no SBUF DMAs),
           v transposed on PE into vaug [s-part, 128 v-cols + ones col].
  Phase 2  per query block (512) x head: scores^T [sk part, sq free],
           exp on scalar engine, causal diagonal handled by a post-exp
           0/1 triangle multiply on gpsimd, then PV with e as stationary
           operand: out[sq, 129] = e^T @ [v | 1] -- the 129th column is
           the softmax denominator (free).  Per-partition reciprocal +
           tensor_scalar normalize, PE transpose back to attnT [d, s].
  Phase 3  o-proj (contraction over this core's 512 head-dims), DMA
           straight from PSUM.  o-proj chunks of query block qb are
           interleaved into the attention j-loops of qb+1 so the PE
           always has work while the scalar engine churns exps.
"""

import contextlib
import math
import numpy as np
import ml_dtypes

import concourse.bacc as bacc
import concourse.mybir as mybir
import concourse.tile as tile
from concourse.bass_utils import run_bass_kernel_spmd

F32 = mybir.dt.float32
BF16 = mybir.dt.bfloat16
AF = mybir.ActivationFunctionType

S = 2048
H = 2048
D = 128            # head dim
KT = 16            # contraction tiles over hidden (2048/128)
NQ = 512           # query block width in attention
SCALE = 1.0 / math.sqrt(D)

_CACHED = {}


def build_nc(loop_n=None):
    nc = bacc.Bacc(None, target_bir_lowering=False)
    hT = nc.dram_tensor("hT", [H, S], BF16, kind="ExternalInput")
    wqk = nc.dram_tensor("wqk", [6, 128, KT * 128], BF16, kind="ExternalInput")
    cosT = nc.dram_tensor("cosT", [D, S], BF16, kind="ExternalInput")
    sinT = nc.dram_tensor("sinT", [D, S], F32, kind="ExternalInput")
    rotp = nc.dram_tensor("rotp", [128, 128], BF16, kind="ExternalInput")
    identb = nc.dram_tensor("identb", [128, 128], BF16, kind="ExternalInput")
    trimask = nc.dram_tensor("trimask", [128, 128], BF16, kind="ExternalInput")
    wo = nc.dram_tensor("wo", [4, 128, H], BF16, kind="ExternalInput")
    out = nc.dram_tensor("out", [S, H], BF16, kind="ExternalOutput")

    with tile.TileContext(nc) as tc:
        with tc.tile_pool(name="persist", bufs=1) as pp:
          with (tc.For_i(0, loop_n, 1) if loop_n else contextlib.nullcontext()):
            # ---- persistent tiles ----
            qk = [pp.tile([128, S], BF16, name=f"qk{i}", tag=f"qk{i}") for i in range(5)]
            vaug = pp.tile([128, 16 * 129], BF16, tag="vaug")
            cos_sb = pp.tile([128, S], BF16, tag="cos")
            sin_sb = pp.tile([128, S], F32, tag="sin")
            rotp_sb = pp.tile([128, 128], BF16, tag="rotp")
            ident_sb = pp.tile([128, 128], BF16, tag="ident")
            tri_sb = pp.tile([128, 128], BF16, tag="tri")
            attnT = [
                pp.tile([128, S], BF16, name=f"at{h}", tag=f"at{h}") for h in range(4)
            ]
            wo_sb = [
                pp.tile([128, H], BF16, name=f"wo{kb}", tag=f"wo{kb}") for kb in range(4)
            ]

            nc.vector.memset(vaug[:, 128::129], 1.0)

            # ---- Phase 1: QKV + RoPE ----
            with (
                tc.tile_pool(name="ht", bufs=2) as htp,
                tc.tile_pool(name="wq", bufs=1) as wqp,
                tc.tile_pool(name="p1sb", bufs=2) as sb1,
                tc.tile_pool(name="psq", bufs=6, space="PSUM") as psq,
                tc.tile_pool(name="psrot", bufs=1, space="PSUM") as psrot,
                tc.tile_pool(name="psvt", bufs=1, space="PSUM") as psvt,
            ):
                w_sb = [
                    wqp.tile([128, KT * 128], BF16, name=f"w{i}", tag=f"w{i}")
                    for i in range(6)
                ]
                for half in range(2):
                    s0 = half * 1024
                    ht = [
                        htp.tile([128, 1024], BF16, name=f"ht{kt}", tag=f"ht{kt}")
                        for kt in range(KT)
                    ]
                    # DMA order (half 0): w0, then ht in consumption order
                    # with the remaining weights spread between; small tiles
                    # and cos/sin (not PE-blocking) late.
                    def dma_ht(kt):
                        nc.sync.dma_start(
                            out=ht[kt][:], in_=hT[kt * 128 : (kt + 1) * 128, s0 : s0 + 1024]
                        )

                    arrival = []  # (kind, idx) matching the DMA queue order
                    if half == 0:
                        nc.sync.dma_start(
                            out=w_sb[0][:, : 8 * 128], in_=wqk[0][:, : 8 * 128]
                        )
                        nc.sync.dma_start(
                            out=w_sb[0][:, 8 * 128 :], in_=wqk[0][:, 8 * 128 :]
                        )
                        arrival.append(("w", 0))
                        for kt in range(KT):
                            dma_ht(kt)
                            arrival.append(("ht", kt))
                            if kt % 3 == 2 and kt // 3 < 5:
                                j = kt // 3 + 1
                                nc.sync.dma_start(out=w_sb[j][:], in_=wqk[j])
                                arrival.append(("w", j))
                        # non-PE-blocking small tiles after the matmul feed
                        nc.sync.dma_start(out=rotp_sb[:], in_=rotp[:])
                        nc.sync.dma_start(out=ident_sb[:], in_=identb[:])
                        nc.sync.dma_start(out=tri_sb[:], in_=trimask[:])
                        nc.sync.dma_start(out=cos_sb[:], in_=cosT[:])
                        nc.sync.dma_start(out=sin_sb[:], in_=sinT[:])
                    else:
                        for kt in range(KT):
                            dma_ht(kt)
                    for c2 in range(2):
                        cs = s0 + c2 * 512
                        ps = [
                            psq.tile([128, 512], F32, name=f"psq{i}", tag="psq")
                            for i in range(6)
                        ]

                        def mm(i, kt):
                            nc.tensor.matmul(
                                ps[i][:],
                                lhsT=w_sb[i][:, kt * 128 : (kt + 1) * 128],
                                rhs=ht[kt][:, c2 * 512 : (c2 + 1) * 512],
                                start=(kt == 0),
                                stop=(kt == KT - 1),
                            )

                        if half == 0 and c2 == 0:
                            # wavefront emission matching DMA arrival order
                            aw, akt = 0, 0
                            for kind, idx in arrival:
                                if kind == "w":
                                    aw = idx + 1
                                    for kt in range(akt):
                                        mm(idx, kt)
                                else:
                                    akt = idx + 1
                                    for i in range(aw):
                                        mm(i, idx)
                        else:
                            # kt-outer: all tiles resident (or streaming ahead)
                            for kt in range(KT):
                                for i in range(6):
                                    mm(i, kt)
                        for i in range(6):
                            _rope_or_v(
                                nc, tc, sb1, psrot, psvt, ps, i, cs,
                                qk, vaug, cos_sb, sin_sb, rotp_sb, ident_sb,
                            )

            # wo weights (needed from first o-proj)
            for kb in range(4):
                nc.sync.dma_start(out=wo_sb[kb][:], in_=wo[kb])

            # ---- Phase 2+3: attention with interleaved o-proj ----
            with (
                tc.tile_pool(name="sps", bufs=3, space="PSUM") as spsp,
                tc.tile_pool(name="pv", bufs=1, space="PSUM") as pvp,
                tc.tile_pool(name="psT", bufs=1, space="PSUM") as psTp,
                tc.tile_pool(name="pso", bufs=2, space="PSUM") as psop,
                tc.tile_pool(name="epool", bufs=18) as ep,
                tc.tile_pool(name="small", bufs=2) as sp,
            ):
                pending = []  # o-proj chunks (sb, n) ready to emit
                ndrained = [0]

                def emit_oproj_chunk(tail=False):
                    sb, n = pending.pop(0)
                    pst = psop.tile([128, 512], F32, tag="po")
                    for kb in range(4):
                        nc.tensor.matmul(
                            pst[:],
                            lhsT=attnT[kb][:, sb * 128 : (sb + 1) * 128],
                            rhs=wo_sb[kb][:, n * 512 : (n + 1) * 512],
                            start=(kb == 0),
                            stop=(kb == 3),
                        )
                    osb = sp.tile([128, 512], BF16, tag="osb")
                    ndrained[0] += 1
                    if tail and ndrained[0] % 2 == 0:
                        nc.scalar.copy(osb[:], pst[:])
                    else:
                        nc.vector.tensor_copy(osb[:], pst[:])
                    nc.sync.dma_start(
                        out=out[sb * 128 : (sb + 1) * 128, n * 512 : (n + 1) * 512],
                        in_=osb[:],
                    )

                kT = qk[4]
                # deficit pacer: emitted-work clocks (ns) for PE vs scalar
                clock = {"pe": 0.0, "act": 0.0}

                def tick(pe_ns=0.0, act_ns=0.0):
                    clock["pe"] += pe_ns
                    clock["act"] += act_ns
                    while pending and clock["act"] > clock["pe"] + 400.0:
                        emit_oproj_chunk()
                        clock["pe"] += 860.0

                for qb in range(4):
                    q0 = qb * NQ
                    nj = 4 * qb + 4
                    for h in range(4):
                        qT = qk[h]
                        e_tiles = {}

                        def emit_j(j):
                            r4 = j - 4 * qb
                            off = max(0, r4) * 128
                            w = NQ - off
                            sps = spsp.tile([128, NQ], F32, tag="sc")
                            nc.tensor.matmul(
                                sps[:, off:NQ],
                                lhsT=kT[:, j * 128 : (j + 1) * 128],
                                rhs=qT[:, q0 + off : q0 + NQ],
                                start=True,
                                stop=True,
                            )
                            e = ep.tile([128, NQ], BF16, tag="e")
                            nc.scalar.activation(
                                e[:, off:NQ], sps[:, off:NQ], AF.Exp, scale=SCALE
                            )
                            if r4 >= 0:
                                nc.gpsimd.tensor_mul(
                                    e[:, off : off + 128],
                                    e[:, off : off + 128],
                                    tri_sb[:],
                                )
                            e_tiles[j] = e
                            tick(pe_ns=w / 2.4, act_ns=w * 0.8333 + 190.0)

                        # scores + exp for all j (o-proj chunks fill the PE
                        # whenever the emitted scalar work runs ahead)
                        for j in range(nj):
                            emit_j(j)

                        # PV: sequential per-t accumulation chains (one open
                        # psum group per bank); the ones-column of vaug makes
                        # column 128 the softmax denominator
                        rec = sp.tile([128, 4], F32, tag="rec")
                        asb = sp.tile([128, NQ], BF16, tag="asb")
                        psT = psTp.tile([128, NQ], BF16, tag="psT")
                        for t in range(4):
                            pvt = pvp.tile([128, 129], F32, tag="pvt")
                            for j in range(4 * qb + t + 1):
                                nc.tensor.matmul(
                                    pvt[:],
                                    lhsT=e_tiles[j][:, t * 128 : (t + 1) * 128],
                                    rhs=vaug[:, j * 129 : (j + 1) * 129],
                                    start=(j == 0),
                                    stop=(j == 4 * qb + t),
                                )
                            tick(pe_ns=(4 * qb + t + 1) * 110.0)
                            nc.vector.reciprocal(rec[:, t : t + 1], pvt[:, 128:129])
                            nc.vector.tensor_scalar_mul(
                                asb[:, t * 128 : (t + 1) * 128],
                                pvt[:, 0:128],
                                rec[:, t : t + 1],
                            )
                            nc.tensor.transpose(
                                psT[:, t * 128 : (t + 1) * 128],
                                asb[:, t * 128 : (t + 1) * 128],
                                ident_sb[:],
                            )
                        nc.vector.tensor_copy(attnT[h][:, q0 : q0 + NQ], psT[:])
                        e_tiles.clear()

                    # queue this qb's o-proj chunks
                    for sbl in range(4):
                        for n in range(4):
                            pending.append((qb * 4 + sbl, n))
                    if qb == 3:
                        while pending:
                            emit_oproj_chunk(tail=True)

    nc.compile()
    return nc


def _rope_or_v(nc, tc, sb1, psrot, psvt, ps, i, cs, qk, vaug, cos_sb, sin_sb,
               rotp_sb, ident_sb):
    """Consume projection psum ps[i] for s-chunk [cs, cs+512)."""
    p = ps[i]
    if i < 5:
        qraw = sb1.tile([128, 512], BF16, tag="qraw")
        # alternate psum-drain copies between scalar and vector engines so
        # the 6 per-chunk drains don't serialize on one engine
        if i % 2 == 0:
            nc.scalar.copy(qraw[:], p[:])
        else:
            nc.vector.tensor_copy(qraw[:], p[:])
        rot = psrot.tile([128, 512], F32, tag="rot")
        nc.tensor.matmul(rot[:], lhsT=rotp_sb[:], rhs=qraw[:], start=True, stop=True)
        tmp = sb1.tile([128, 512], F32, tag="tmp")
        nc.vector.tensor_mul(tmp[:], rot[:], sin_sb[:, cs : cs + 512])
        t2 = sb1.tile([128, 512], F32, tag="t2")
        nc.gpsimd.tensor_mul(t2[:], qraw[:], cos_sb[:, cs : cs + 512])
        nc.vector.tensor_add(qk[i][:, cs : cs + 512], t2[:], tmp[:])
    else:
        vTc = sb1.tile([128, 512], BF16, tag="vTc")
        nc.vector.tensor_copy(vTc[:], p[:])
        for sbl in range(4):
            sb = cs // 128 + sbl
            pv = psvt.tile([128, 128], BF16, tag="psv")
            nc.tensor.transpose(pv[:], vTc[:, sbl * 128 : (sbl + 1) * 128], ident_sb[:])
            nc.scalar.copy(vaug[:, sb * 129 : sb * 129 + 128], pv[:])


def _prep_inputs(hidden_states, cos, sin, w_qkv, w_o):
    """Build the 8 per-core input maps (host-side shard + transpose)."""
    bf = ml_dtypes.bfloat16
    hidden_states = np.asarray(hidden_states, dtype=np.float32)
    cos = np.asarray(cos, dtype=np.float32)
    sin = np.asarray(sin, dtype=np.float32)
    w_qkv = np.asarray(w_qkv, dtype=np.float32)
    w_o = np.asarray(w_o, dtype=np.float32)

    cosT = np.ascontiguousarray(cos.T).astype(bf)
    sinT = np.ascontiguousarray(sin.T).copy()
    sinT[0:64] *= -1.0  # rotate_half sign folded into sin

    d = np.arange(128)
    rotp = ((d[None, :] == (d[:, None] + 64) % 128)).astype(bf)
    identb = np.eye(128).astype(bf)
    sk = np.arange(128)[:, None]
    c = np.arange(128)[None, :]
    trimask = (sk <= c).astype(bf)

    hT = [np.ascontiguousarray(hidden_states[b].T).astype(bf) for b in range(2)]

    in_maps = []
    for cidx in range(8):
        b, g = divmod(cidx, 4)
        W6 = np.stack(
            [w_qkv[(4 * g + i) * 128 : (4 * g + i + 1) * 128] for i in range(4)]
            + [w_qkv[(16 + g) * 128 : (17 + g) * 128]]
            + [w_qkv[(20 + g) * 128 : (21 + g) * 128]]
        )  # [6, 128 m, 2048 h]
        wqk_pack = np.ascontiguousarray(
            W6.transpose(0, 2, 1)  # [6, h, m]
            .reshape(6, KT, 128, 128)  # [6, kt, p, m]
            .transpose(0, 2, 1, 3)  # [6, p, kt, m]
            .reshape(6, 128, KT * 128)
        ).astype(bf)
        wo_pack = np.ascontiguousarray(
            np.stack(
                [
                    w_o[:, (4 * g + kb) * 128 : (4 * g + kb + 1) * 128].T
                    for kb in range(4)
                ]
            )
        ).astype(bf)  # [4, 128 hd, 2048 o]
        in_maps.append(
            dict(
                hT=hT[b],
                wqk=wqk_pack,
                cosT=cosT,
                sinT=sinT,
                rotp=rotp,
                identb=identb,
                trimask=trimask,
                wo=wo_pack,
            )
        )
    return in_maps


def run(hidden_states, cos, sin, w_qkv, w_o, trace=False, **trace_kwargs):
    if "nc" not in _CACHED:
        _CACHED["nc"] = build_nc()
    nc = _CACHED["nc"]
    in_maps = _prep_inputs(hidden_states, cos, sin, w_qkv, w_o)
    res = run_bass_kernel_spmd(
        nc, in_maps, core_ids=list(range(8)), trace=trace, **trace_kwargs
    )
    outs = [res.results[c]["out"].astype(np.float32) for c in range(8)]
    full = np.stack(
        [
            outs[0] + outs[1] + outs[2] + outs[3],
            outs[4] + outs[5] + outs[6] + outs[7],
        ]
    ).astype(np.float32)
    return full, res


def kernel(hidden_states, cos, sin, w_qkv, w_o):
    full, _ = run(hidden_states, cos, sin, w_qkv, w_o, trace=False)
    return full


# revision 23
# speedup vs baseline: 3.8931x; 3.8931x over previous
"""Trainium2 Bass kernel for GQA attention block (B=2, S=2048, H=2048,
16 q-heads / 4 kv-heads, head_dim=128, RoPE, causal) on 8 NeuronCores.

Sharding: core c -> batch b = c // 4, kv-group g = c % 4
  (q heads 4g..4g+3, kv head g).  Each core computes its batch's
  attention for its 4 query heads plus the partial output projection
  over its 512 hidden columns of w_o; host sums the 4 partials per batch.

Design (all matmuls bf16, PSUM accumulate f32):
  Phase 1  QKV projection in s-chunks of 512 (i-outer, kt-inner chains),
           RoPE via PE permutation matmul for rotate-half (no SBUF DMAs),
           v transposed on PE into vaug [s-part, 128 v-cols + ones col].
  Phase 2  per query block (512) x head: scores^T [sk part, sq free],
           exp on scalar engine, causal diagonal handled by a post-exp
           0/1 triangle multiply on gpsimd, then PV with e as stationary
           operand: out[sq, 129] = e^T @ [v | 1] -- the 129th column is
           the softmax denominator (free).  Per-partition reciprocal +
           tensor_scalar normalize, PE transpose back to attnT [d, s].
  Phase 3  o-proj (contraction over this core's 512 head-dims), DMA
           straight from PSUM.  o-proj chunks of query block qb are
           interleaved into the attention j-loops of qb+1 so the PE
           always has work while the scalar engine churns exps.
"""

import contextlib
import math
import numpy as np
import ml_dtypes

import concourse.bacc as bacc
import concourse.mybir as mybir
import concourse.tile as tile
from concourse.bass_utils import run_bass_kernel_spmd

F32 = mybir.dt.float32
BF16 = mybir.dt.bfloat16
AF = mybir.ActivationFunctionType

S = 2048
H = 2048
D = 128            # head dim
KT = 16            # contraction tiles over hidden (2048/128)
NQ = 512           # query block width in attention
SCALE = 1.0 / math.sqrt(D)

_CACHED = {}


def build_nc(loop_n=None):
    nc = bacc.Bacc(None, target_bir_lowering=False)
    hT = nc.dram_tensor("hT", [H, S], BF16, kind="ExternalInput")
    wqk = nc.dram_tensor("wqk", [6, 128, KT * 128], BF16, kind="ExternalInput")
    cosT = nc.dram_tensor("cosT", [D, S], BF16, kind="ExternalInput")
    sinT = nc.dram_tensor("sinT", [D, S], F32, kind="ExternalInput")
    rotp = nc.dram_tensor("rotp", [128, 128], BF16, kind="ExternalInput")
    identb = nc.dram_tensor("identb", [128, 128], BF16, kind="ExternalInput")
    trimask = nc.dram_tensor("trimask", [128, 128], BF16, kind="ExternalInput")
    wo = nc.dram_tensor("wo", [4, 128, H], BF16, kind="ExternalInput")
    out = nc.dram_tensor("out", [S, H], BF16, kind="ExternalOutput")

    with tile.TileContext(nc) as tc:
        with tc.tile_pool(name="persist", bufs=1) as pp:
          with (tc.For_i(0, loop_n, 1) if loop_n else contextlib.nullcontext()):
            # ---- persistent tiles ----
            qk = [pp.tile([128, S], BF16, name=f"qk{i}", tag=f"qk{i}") for i in range(5)]
            vaug = pp.tile([128, 16 * 129], BF16, tag="vaug")
            cos_sb = pp.tile([128, S], BF16, tag="cos")
            sin_sb = pp.tile([128, S], F32, tag="sin")
            rotp_sb = pp.tile([128, 128], BF16, tag="rotp")
            ident_sb = pp.tile([128, 128], BF16, tag="ident")
            tri_sb = pp.tile([128, 128], BF16, tag="tri")
            attnT = [
                pp.tile([128, S], BF16, name=f"at{h}", tag=f"at{h}") for h in range(4)
            ]
            wo_sb = [
                pp.tile([128, H], BF16, name=f"wo{kb}", tag=f"wo{kb}") for kb in range(4)
            ]

            nc.vector.memset(vaug[:, 128::129], 1.0)

            # ---- Phase 1: QKV + RoPE ----
            with (
                tc.tile_pool(name="ht", bufs=2) as htp,
                tc.tile_pool(name="wq", bufs=1) as wqp,
                tc.tile_pool(name="p1sb", bufs=2) as sb1,
                tc.tile_pool(name="psq", bufs=6, space="PSUM") as psq,
                tc.tile_pool(name="psrot", bufs=1, space="PSUM") as psrot,
                tc.tile_pool(name="psvt", bufs=1, space="PSUM") as psvt,
            ):
                w_sb = [
                    wqp.tile([128, KT * 128], BF16, name=f"w{i}", tag=f"w{i}")
                    for i in range(6)
                ]
                for half in range(2):
                    s0 = half * 1024
                    ht = [
                        htp.tile([128, 1024], BF16, name=f"ht{kt}", tag=f"ht{kt}")
                        for kt in range(KT)
                    ]
                    # DMA order (half 0): w0, then ht in consumption order
                    # with the remaining weights spread between; small tiles
                    # and cos/sin (not PE-blocking) late.
                    def dma_ht(kt):
                        nc.sync.dma_start(
                            out=ht[kt][:], in_=hT[kt * 128 : (kt + 1) * 128, s0 : s0 + 1024]
                        )

                    arrival = []  # (kind, idx) matching the DMA queue order
                    if half == 0:
                        nc.sync.dma_start(out=w_sb[0][:], in_=wqk[0])
                        arrival.append(("w", 0))
                        for kt in range(KT):
                            dma_ht(kt)
                            arrival.append(("ht", kt))
                            if kt % 3 == 2 and kt // 3 < 5:
                                j = kt // 3 + 1
                                nc.sync.dma_start(out=w_sb[j][:], in_=wqk[j])
                                arrival.append(("w", j))
                        # non-PE-blocking small tiles after the matmul feed
                        nc.sync.dma_start(out=rotp_sb[:], in_=rotp[:])
                        nc.sync.dma_start(out=ident_sb[:], in_=identb[:])
                        nc.sync.dma_start(out=tri_sb[:], in_=trimask[:])
                        nc.sync.dma_start(out=cos_sb[:], in_=cosT[:])
                        nc.sync.dma_start(out=sin_sb[:], in_=sinT[:])
                    else:
                        for kt in range(KT):
                            dma_ht(kt)
                    for c2 in range(2):
                        cs = s0 + c2 * 512
                        ps = [
                            psq.tile([128, 512], F32, name=f"psq{i}", tag="psq")
                            for i in range(6)
                        ]

                        def mm(i, kt):
                            nc.tensor.matmul(
                                ps[i][:],
                                lhsT=w_sb[i][:, kt * 128 : (kt + 1) * 128],
                                rhs=ht[kt][:, c2 * 512 : (c2 + 1) * 512],
                                start=(kt == 0),
                                stop=(kt == KT - 1),
                            )

                        if half == 0 and c2 == 0:
                            # wavefront emission matching DMA arrival order
                            aw, akt = 0, 0
                            for kind, idx in arrival:
                                if kind == "w":
                                    aw = idx + 1
                                    for kt in range(akt):
                                        mm(idx, kt)
                                else:
                                    akt = idx + 1
                                    for i in range(aw):
                                        mm(i, idx)
                        else:
                            # kt-outer: all tiles resident (or streaming ahead)
                            for kt in range(KT):
                                for i in range(6):
                                    mm(i, kt)
                        for i in range(6):
                            _rope_or_v(
                                nc, tc, sb1, psrot, psvt, ps, i, cs,
                                qk, vaug, cos_sb, sin_sb, rotp_sb, ident_sb,
                            )

            # wo weights (needed from first o-proj)
            for kb in range(4):
                nc.sync.dma_start(out=wo_sb[kb][:], in_=wo[kb])

            # ---- Phase 2+3: attention with interleaved o-proj ----
            with (
                tc.tile_pool(name="sps", bufs=3, space="PSUM") as spsp,
                tc.tile_pool(name="pv", bufs=1, space="PSUM") as pvp,
                tc.tile_pool(name="psT", bufs=1, space="PSUM") as psTp,
                tc.tile_pool(name="pso", bufs=2, space="PSUM") as psop,
                tc.tile_pool(name="epool", bufs=18) as ep,
                tc.tile_pool(name="small", bufs=2) as sp,
            ):
                pending = []  # o-proj chunks (sb, n) ready to emit
                ndrained = [0]

                def emit_oproj_chunk(tail=False):
                    sb, n = pending.pop(0)
                    pst = psop.tile([128, 512], F32, tag="po")
                    for kb in range(4):
                        nc.tensor.matmul(
                            pst[:],
                            lhsT=attnT[kb][:, sb * 128 : (sb + 1) * 128],
                            rhs=wo_sb[kb][:, n * 512 : (n + 1) * 512],
                            start=(kb == 0),
                            stop=(kb == 3),
                        )
                    osb = sp.tile([128, 512], BF16, tag="osb")
                    nc.vector.tensor_copy(osb[:], pst[:])
                    nc.sync.dma_start(
                        out=out[sb * 128 : (sb + 1) * 128, n * 512 : (n + 1) * 512],
                        in_=osb[:],
                    )

                kT = qk[4]
                # deficit pacer: emitted-work clocks (ns) for PE vs scalar
                clock = {"pe": 0.0, "act": 0.0}

                def tick(pe_ns=0.0, act_ns=0.0):
                    clock["pe"] += pe_ns
                    clock["act"] += act_ns
                    while pending and clock["act"] > clock["pe"] + 400.0:
                        emit_oproj_chunk()
                        clock["pe"] += 860.0

                for qb in range(4):
                    q0 = qb * NQ
                    nj = 4 * qb + 4
                    for h in range(4):
                        qT = qk[h]
                        e_tiles = {}

                        def emit_j(j):
                            r4 = j - 4 * qb
                            off = max(0, r4) * 128
                            w = NQ - off
                            sps = spsp.tile([128, NQ], F32, tag="sc")
                            nc.tensor.matmul(
                                sps[:, off:NQ],
                                lhsT=kT[:, j * 128 : (j + 1) * 128],
                                rhs=qT[:, q0 + off : q0 + NQ],
                                start=True,
                                stop=True,
                            )
                            e = ep.tile([128, NQ], BF16, tag="e")
                            nc.scalar.activation(
                                e[:, off:NQ], sps[:, off:NQ], AF.Exp, scale=SCALE
                            )
                            if r4 >= 0:
                                nc.gpsimd.tensor_mul(
                                    e[:, off : off + 128],
                                    e[:, off : off + 128],
                                    tri_sb[:],
                                )
                            e_tiles[j] = e
                            tick(pe_ns=w / 2.4, act_ns=w * 0.8333 + 190.0)

                        # scores + exp for all j (o-proj chunks fill the PE
                        # whenever the emitted scalar work runs ahead)
                        for j in range(nj):
                            emit_j(j)

                        # PV: sequential per-t accumulation chains (one open
                        # psum group per bank); the ones-column of vaug makes
                        # column 128 the softmax denominator
                        rec = sp.tile([128, 4], F32, tag="rec")
                        asb = sp.tile([128, NQ], BF16, tag="asb")
                        psT = psTp.tile([128, NQ], BF16, tag="psT")
                        for t in range(4):
                            pvt = pvp.tile([128, 129], F32, tag="pvt")
                            for j in range(4 * qb + t + 1):
                                nc.tensor.matmul(
                                    pvt[:],
                                    lhsT=e_tiles[j][:, t * 128 : (t + 1) * 128],
                                    rhs=vaug[:, j * 129 : (j + 1) * 129],
                                    start=(j == 0),
                                    stop=(j == 4 * qb + t),
                                )
                            tick(pe_ns=(4 * qb + t + 1) * 110.0)
                            nc.vector.reciprocal(rec[:, t : t + 1], pvt[:, 128:129])
                            nc.vector.tensor_scalar_mul(
                                asb[:, t * 128 : (t + 1) * 128],
                                pvt[:, 0:128],
                                rec[:, t : t + 1],
                            )
                            nc.tensor.transpose(
                                psT[:, t * 128 : (t + 1) * 128],
                                asb[:, t * 128 : (t + 1) * 128],
                                ident_sb[:],
                            )
                        nc.vector.tensor_copy(attnT[h][:, q0 : q0 + NQ], psT[:])
                        e_tiles.clear()

                    # queue this qb's o-proj chunks
                    for sbl in range(4):
                        for n in range(4):
                            pending.append((qb * 4 + sbl, n))
                    if qb == 3:
                        while pending:
                            emit_oproj_chunk(tail=True)

    nc.compile()
    return nc


def _rope_or_v(nc, tc, sb1, psrot, psvt, ps, i, cs, qk, vaug, cos_sb, sin_sb,
               rotp_sb, ident_sb):
    """Consume projection psum ps[i] for s-chunk [cs, cs+512)."""
    p = ps[i]
    if i < 5:
        qraw = sb1.tile([128, 512], BF16, tag="qraw")
        # alternate psum-drain copies between scalar and vector engines so
        # the 6 per-chunk drains don't serialize on one engine
        if i % 2 == 0:
            nc.scalar.copy(qraw[:], p[:])
        else:
            nc.vector.tensor_copy(qraw[:], p[:])
        rot = psrot.tile([128, 512], F32, tag="rot")
        nc.tensor.matmul(rot[:], lhsT=rotp_sb[:], rhs=qraw[:], start=True, stop=True)
        tmp = sb1.tile([128, 512], F32, tag="tmp")
        nc.vector.tensor_mul(tmp[:], rot[:], sin_sb[:, cs : cs + 512])
        t2 = sb1.tile([128, 512], F32, tag="t2")
        nc.gpsimd.tensor_mul(t2[:], qraw[:], cos_sb[:, cs : cs + 512])
        nc.vector.tensor_add(qk[i][:, cs : cs + 512], t2[:], tmp[:])
    else:
        vTc = sb1.tile([128, 512], BF16, tag="vTc")
        nc.vector.tensor_copy(vTc[:], p[:])
        for sbl in range(4):
            sb = cs // 128 + sbl
            pv = psvt.tile([128, 128], BF16, tag="psv")
            nc.tensor.transpose(pv[:], vTc[:, sbl * 128 : (sbl + 1) * 128], ident_sb[:])
            nc.scalar.copy(vaug[:, sb * 129 : sb * 129 + 128], pv[:])


def _prep_inputs(hidden_states, cos, sin, w_qkv, w_o):
    """Build the 8 per-core input maps (host-side shard + transpose)."""
    bf = ml_dtypes.bfloat16
    hidden_states = np.asarray(hidden_states, dtype=np.float32)
    cos = np.asarray(cos, dtype=np.float32)
    sin = np.asarray(sin, dtype=np.float32)
    w_qkv = np.asarray(w_qkv, dtype=np.float32)
    w_o = np.asarray(w_o, dtype=np.float32)

    cosT = np.ascontiguousarray(cos.T).astype(bf)
    sinT = np.ascontiguousarray(sin.T).copy()
    sinT[0:64] *= -1.0  # rotate_half sign folded into sin

    d = np.arange(128)
    rotp = ((d[None, :] == (d[:, None] + 64) % 128)).astype(bf)
    identb = np.eye(128).astype(bf)
    sk = np.arange(128)[:, None]
    c = np.arange(128)[None, :]
    trimask = (sk <= c).astype(bf)

    hT = [np.ascontiguousarray(hidden_states[b].T).astype(bf) for b in range(2)]

    in_maps = []
    for cidx in range(8):
        b, g = divmod(cidx, 4)
        W6 = np.stack(
            [w_qkv[(4 * g + i) * 128 : (4 * g + i + 1) * 128] for i in range(4)]
            + [w_qkv[(16 + g) * 128 : (17 + g) * 128]]
            + [w_qkv[(20 + g) * 128 : (21 + g) * 128]]
        )  # [6, 128 m, 2048 h]
        wqk_pack = np.ascontiguousarray(
            W6.transpose(0, 2, 1)  # [6, h, m]
            .reshape(6, KT, 128, 128)  # [6, kt, p, m]
            .transpose(0, 2, 1, 3)  # [6, p, kt, m]
            .reshape(6, 128, KT * 128)
        ).astype(bf)
        wo_pack = np.ascontiguousarray(
            np.stack(
                [
                    w_o[:, (4 * g + kb) * 128 : (4 * g + kb + 1) * 128].T
                    for kb in range(4)
                ]
            )
        ).astype(bf)  # [4, 128 hd, 2048 o]
        in_maps.append(
            dict(
                hT=hT[b],
                wqk=wqk_pack,
                cosT=cosT,
                sinT=sinT,
                rotp=rotp,
                identb=identb,
                trimask=trimask,
                wo=wo_pack,
            )
        )
    return in_maps


def run(hidden_states, cos, sin, w_qkv, w_o, trace=False, **trace_kwargs):
    if "nc" not in _CACHED:
        _CACHED["nc"] = build_nc()
    nc = _CACHED["nc"]
    in_maps = _prep_inputs(hidden_states, cos, sin, w_qkv, w_o)
    res = run_bass_kernel_spmd(
        nc, in_maps, core_ids=list(range(8)), trace=trace, **trace_kwargs
    )
    outs = [res.results[c]["out"].astype(np.float32) for c in range(8)]
    full = np.stack(
        [
            outs[0] + outs[1] + outs[2] + outs[3],
            outs[4] + outs[5] + outs[6] + outs[7],
        ]
    ).astype(np.float32)
    return full, res


def kernel(hidden_states, cos, sin, w_qkv, w_o):
    full, _ = run(hidden_states, cos, sin, w_qkv, w_o, trace=False)
    return full
